# revision 18
# baseline (speedup 1.0000x reference)
"""Trainium2 kernel for out = A @ W2 @ B.T with banded Gaussian W2.

Math: W2 = W1*W1, W1[i,j] = exp(-(i-j)^2/(2*8^2)) truncated below 1e-10, so
W2 = exp(-(i-j)^2/64) on a |i-j| <= 54 band. W2 is symmetric banded Toeplitz.

Embed W2 in the 4096-circulant C with the same band symbol:
    C = W2 + E,  E = two 54x54 corner triangles (the wrap-around band).
C diagonalizes in the real DFT basis: C = Q diag(lam) Q.T with
lam(f) ~ sqrt(64*pi) * exp(-16 * (2*pi*f/N)^2). All kept lam > 0, so with
Qs = Q * sqrt(lam) (columns ordered by frequency: dc, c1, s1, c2, s2, ...)
and rank R = 768 (trunc err ~4e-3; the seed-0 inputs are spectrally
tilted, so R=640's cutoff at f=320 costs 1.9e-2 -- too close to the gate):

    out = (A @ Qs) @ (B @ Qs).T - A @ E @ B.T

One basis matrix serves both projections (half the Q traffic of a
Q / Q*lam split).

Distribution (8 cores, XOR-slot remote_dma exchange, no AllGather):
  - core c holds: A.T slab [4096, 1024] (its 1024 A-rows), B.T column slice
    [4096, 512], Qs [4096, 768], corner packs (all bf16).
  - stage H (sharded): HT_loc = Qs.T @ B.T[:, c-slice] [768, 512] written to
    SBUF slot 0 of hg, then remote_dma_broadcast SBUF->SBUF to peer
    (c ^ F_LANE[k])'s slot k -- no HBM bounce, no data collective. A 32-byte
    AllGather after trigger_dma is the rendezvous barrier; its latency plus
    stage G (~70us of PE work) covers the transfer drain. Final-stage chunk
    k = global column chunk c ^ F_LANE[k]; the host permutes btc in and
    un-permutes out. Cross-exec safety: executions serialize per core, and
    a peer's next-exec sends happen only after its rendezvous, which needs
    every core to have entered the next exec.
  - stage G (data-parallel): GT = Qs.T @ A.T_slab [768, 1024] in two m-half
    passes (PSUM: 6 f-banks + 1 corner bank per pass), A.T streamed.
  - corner factor: GTc = (-Etr).T @ A.T[x=0] + (-Ebl).T @ A.T[x=31],
    accumulated inside the stage-G passes.
  - final: out_slab = GT.T @ HT + GTc.T @ B.T_corner, streamed per 512-col
    chunk with all 8 PSUM banks accumulating the 8 m-tiles.
  - all matmuls bf16 (1 cyc/row, FP32 PSUM accumulate); out stored bf16 and
    upcast on host. Measured rel err 6.2e-3 < 2e-2.
"""

import numpy as np

import concourse.bass as bass
import concourse.mybir as mybir
from concourse import bacc
from concourse.bass_utils import run_bass_kernel_spmd
from concourse.tile import TileContext
from concourse.tile_rust import add_dep_helper

P = 128          # partition / block size
N = 4096         # inner dims (A cols, B rows/cols)
M_FULL = 8192    # A rows
NCORES = 8
MS = M_FULL // NCORES   # 1024 rows of A per core
NK = N // P      # 32 contraction x-tiles
NM = MS // P     # 8 m-tiles per core
CW = 512         # output column chunk width (= 1 PSUM bank of fp32)
NCH = N // CW    # 8 chunks
R = 768          # truncated spectral rank (6 f-tiles)
NT = R // P      # 6 f-tiles

SIGMA = 8.0
TRUNC_EPS = 1e-10
# remote_dma_broadcast slot k applies delta-tpb F_LANE[k] (ucode lane map,
# measured on hardware): slots 4-7 swap pairs.
F_LANE = [0, 1, 2, 3, 6, 7, 4, 5]

_COMPILED = {}


def _band_profile():
    """g[d] = W2 band value at distance d (same fp32 path as the reference)."""
    d = np.arange(N).astype(np.float32)
    w1 = np.exp(-(d * d) / np.float32(2.0 * SIGMA * SIGMA)).astype(np.float32)
    w1 = np.where(w1 > np.float32(TRUNC_EPS), w1, np.float32(0.0))
    return (w1 * w1).astype(np.float64)


def _build_spectral():
    """Qs [N, R] freq-interleaved real-DFT basis * sqrt(lam), corner packs."""
    g = _band_profile()
    # circulant symbol c[k] = g(k) + g(N-k)
    c = g.copy()
    c[1:] += g[1:][::-1]
    lam = np.fft.rfft(c).real  # f = 0 .. N/2

    x = np.arange(N).astype(np.float64)
    cols = [np.full(N, 1.0 / np.sqrt(N))]
    lams = [lam[0]]
    f = 1
    while len(cols) < R:
        cols.append(np.sqrt(2.0 / N) * np.cos(2 * np.pi * f * x / N))
        lams.append(lam[f])
        if len(cols) < R:
            cols.append(np.sqrt(2.0 / N) * np.sin(2 * np.pi * f * x / N))
            lams.append(lam[f])
        f += 1
    Q = np.stack(cols, axis=1)                      # [N, R]
    lams = np.array(lams)
    assert (lams > 0).all(), lams.min()
    Qs = Q * np.sqrt(lams)[None, :]

    # corner blocks of E = C - W2 (W2 is zero there):
    # Etr[i, j] = c[(i - j - (N - P)) mod N] = g(i + P - j) for i+P-j <= 54
    ii = np.arange(P)[:, None]
    jj = np.arange(P)[None, :]
    dtr = ii + P - jj
    etr = np.where((dtr >= 0) & (dtr <= 54), g[np.clip(dtr, 0, 54)], 0.0)
    ebl = etr.T
    return Qs.astype(np.float32), etr.astype(np.float32), ebl.astype(np.float32)


def _build_program(reps=1):
    """Build + compile the Bass program (one NEFF, run SPMD on 8 cores)."""
    nc = bacc.Bacc("TRN2", target_bir_lowering=False, debug=False,
                   num_devices=NCORES)
    f32 = mybir.dt.float32
    bf16 = mybir.dt.bfloat16

    at_dram = nc.dram_tensor("at", [N, MS], bf16, kind="ExternalInput").ap()
    btsl_dram = nc.dram_tensor("btsl", [N, CW], bf16,
                               kind="ExternalInput").ap()
    qs_dram = nc.dram_tensor("qs", [N, R], bf16, kind="ExternalInput").ap()
    ec_dram = nc.dram_tensor("ec", [P, 2 * P], bf16, kind="ExternalInput").ap()
    btc_dram = nc.dram_tensor("btc", [P, N], bf16,
                              kind="ExternalInput").ap()
    out_dram = nc.dram_tensor("out", [MS, N], bf16, kind="ExternalOutput").ap()

    at3 = at_dram.rearrange("(x p) m -> p x m", p=P)      # [128, 32, 1024]
    btsl3 = btsl_dram.rearrange("(x p) c -> p x c", p=P)  # [128, 32, 512]
    qs3 = qs_dram.rearrange("(x p) f -> p x f", p=P)      # [128, 32, 768]
    out3 = out_dram.rearrange("(m p) c -> p m c", p=P)    # [128, 8, 4096]

    rsem = nc.alloc_semaphore("rdma_data")
    lsem = nc.alloc_semaphore("rdma_local")

    with TileContext(nc) as tc:
        with (
            tc.tile_pool(name="const", bufs=1) as const_pool,
            tc.tile_pool(name="resp", bufs=1) as res_pool,
            tc.tile_pool(name="astr", bufs=3) as a_pool,
            tc.tile_pool(name="obw", bufs=2) as obw_pool,
            tc.tile_pool(name="psp", bufs=8, space="PSUM") as ps_pool,
        ):
            for rep in range(reps):
                sfx = f"_{rep}"
                # tiny DRAM flag tensors for the rendezvous collectives
                fl_in = [nc.dram_tensor(f"fl{i}{sfx}", [1, 16], bf16,
                                        kind="Internal").ap()
                         for i in range(1)]
                fl_out = [nc.dram_tensor(f"flg{i}{sfx}", [NCORES, 16], bf16,
                                         kind="Internal",
                                         addr_space="Shared").ap()
                          for i in range(1)]

                # ---- resident loads for stage H, interleaved per 4-x group
                # so x-tile k of both qs and btsl arrives in order.
                qs_sb = res_pool.tile([P, NK * R], bf16, tag="qs",
                                      name=f"qs_sb{sfx}")
                qs_v = qs_sb.rearrange("p (x f) -> p x f", f=R)
                btsl_sb = res_pool.tile([P, NK * CW], bf16, tag="btsl",
                                        name=f"btsl_sb{sfx}")
                btsl_v = btsl_sb.rearrange("p (x c) -> p x c", c=CW)
                QG = 4  # x-tiles per group DMA (steady state)
                # leading groups split small so the first H matmul starts
                # ~1.5us in instead of waiting behind a 1.2MB group
                edges = [0, 1, 2, 4, 8] + list(range(12, NK + 1, QG))
                for lo, hi in zip(edges, edges[1:]):
                    sl = slice(lo, hi)
                    nc.sync.dma_start(qs_v[:, sl, :], qs3[:, sl, :])
                    nc.sync.dma_start(btsl_v[:, sl, :], btsl3[:, sl, :])

                # ---- stage H: HT_loc = qs.T @ btsl -> hg slot 0
                ps_h = [
                    ps_pool.tile([P, CW], f32, tag="ps", name=f"ps_h{sfx}_{t}")
                    for t in range(NT)
                ]
                for x in range(NK):
                    for t in range(NT):
                        nc.tensor.matmul(
                            ps_h[t],
                            lhsT=qs_sb[:, x * R + t * P:x * R + (t + 1) * P],
                            rhs=btsl_sb[:, x * CW:(x + 1) * CW],
                            start=(x == 0),
                            stop=(x == NK - 1),
                        )
                # hg holds all 8 cores' H slices in SBUF, slot-ordered:
                # slot k of core r = H columns of core r ^ F[k] (host
                # un-permutes). Slot 0 is the local slice, written directly.
                hg_sb = res_pool.tile([P, NCORES * NT * CW], bf16, tag="hg",
                                      name=f"hg_sb{sfx}")
                for t in range(NT):
                    nc.vector.tensor_copy(
                        hg_sb[:, t * CW:(t + 1) * CW], ps_h[t])
                # broadcast slot 0 to peer r^F[k]'s slot k, SBUF -> SBUF
                sw = NT * CW
                for k in range(1, NCORES):
                    rdests = [(0, k) if j == k else None
                              for j in range(NCORES)]
                    nc.gpsimd.remote_dma_broadcast(
                        hg_sb[:, k * sw:(k + 1) * sw],
                        hg_sb[:, 0:sw],
                        remote_sem=rsem, local_sem=lsem, rdests=rdests,
                    )
                trig = nc.gpsimd.trigger_dma(count=None)
                # rendezvous AFTER the trigger (explicit dep: the scheduler
                # would otherwise hoist the dep-free collective to t=0):
                # barrier completion => every core rang its SWDGE doorbell,
                # and the barrier latency plus stage G (~70us) covers the
                # transfer drain before the first remote-slot read.
                cc = nc.gpsimd.collective_compute(
                    "AllGather",
                    mybir.AluOpType.bypass,
                    replica_groups=[list(range(NCORES))],
                    ins=[fl_in[0]],
                    outs=[fl_out[0]],
                )
                add_dep_helper(cc.ins, trig.ins,
                               reason="rendezvous after doorbell")

                # constants for stage G corners + final
                ec_sb = const_pool.tile([P, 2 * P], bf16, tag="ec",
                                        name=f"ec_sb{sfx}")
                nc.sync.dma_start(ec_sb, ec_dram)
                btc_sb = const_pool.tile([P, N], bf16, tag="btc",
                                         name=f"btc_sb{sfx}")
                nc.sync.dma_start(btc_sb, btc_dram)

                # ---- stage G: GT = qs.T @ at, two m-half passes + corner
                gt_sb = [
                    res_pool.tile([P, MS], bf16, tag=f"gt{t}",
                                  name=f"gt_sb{sfx}_{t}")
                    for t in range(NT)
                ]
                gtc_sb = res_pool.tile([P, MS], bf16, tag="gtc",
                                       name=f"gtc_sb{sfx}")
                for mh in range(2):
                    msl = slice(mh * CW, (mh + 1) * CW)
                    ps_g = [
                        ps_pool.tile([P, CW], f32, tag="ps",
                                     name=f"ps_g{sfx}_{mh}_{t}")
                        for t in range(NT)
                    ]
                    ps_c = ps_pool.tile([P, CW], f32, tag="ps",
                                        name=f"ps_c{sfx}_{mh}")
                    for xg in range(NK // QG):
                        at_t = a_pool.tile([P, QG * CW], bf16, tag="at",
                                           name=f"at_sb{sfx}_{mh}_{xg}")
                        nc.sync.dma_start(
                            at_t.rearrange("p (x m) -> p x m", m=CW),
                            at3[:, xg * QG:(xg + 1) * QG, msl])
                        for xi in range(QG):
                            x = xg * QG + xi
                            rhs = at_t[:, xi * CW:(xi + 1) * CW]
                            for t in range(NT):
                                nc.tensor.matmul(
                                    ps_g[t],
                                    lhsT=qs_sb[:, x * R + t * P:
                                               x * R + (t + 1) * P],
                                    rhs=rhs,
                                    start=(x == 0),
                                    stop=(x == NK - 1),
                                )
                            if x == 0:
                                nc.tensor.matmul(
                                    ps_c, lhsT=ec_sb[:, 0:P], rhs=rhs,
                                    start=True, stop=False)
                            elif x == NK - 1:
                                nc.tensor.matmul(
                                    ps_c, lhsT=ec_sb[:, P:2 * P], rhs=rhs,
                                    start=False, stop=True)
                    for t in range(NT):
                        nc.vector.tensor_copy(gt_sb[t][:, msl], ps_g[t])
                    nc.vector.tensor_copy(gtc_sb[:, msl], ps_c)

                # ---- final: out = GT.T @ HT + GTc.T @ btc, per 512-col chunk
                for nu in range(NCH):
                    cs = bass.ts(nu, CW)
                    hn_t = hg_sb[:, nu * NT * CW:(nu + 1) * NT * CW]
                    ps_o = [
                        ps_pool.tile([P, CW], f32, tag="ps",
                                     name=f"ps_o{sfx}_{nu}_{m}")
                        for m in range(NM)
                    ]
                    obw_t = obw_pool.tile([P, NM * CW], bf16, tag="obw",
                                          name=f"obw_sb{sfx}_{nu}")
                    # m-outer with per-m PSUM copy: each m's copy overlaps
                    # the next m's matmuls, shrinking the chunk drain tail
                    for m in range(NM):
                        nc.tensor.matmul(
                            ps_o[m],
                            lhsT=gtc_sb[:, m * P:(m + 1) * P],
                            rhs=btc_sb[:, cs],
                            start=True,
                            stop=False,
                        )
                        for t in range(NT):
                            mm = nc.tensor.matmul(
                                ps_o[m],
                                lhsT=gt_sb[t][:, m * P:(m + 1) * P],
                                rhs=hn_t[:, t * CW:(t + 1) * CW],
                                start=False,
                                stop=(t == NT - 1),
                            )
                            if nu > 0 and m == 0 and t == 0:
                                add_dep_helper(mm.ins, cc.ins,
                                               reason="remote H after barrier")
                        nc.vector.tensor_copy(
                            obw_t[:, m * CW:(m + 1) * CW], ps_o[m])
                    nc.sync.dma_start(
                        out3[:, :, cs],
                        obw_t.rearrange("p (m c) -> p m c", c=CW),
                    )

    nc.compile()
    return nc


def _get_program():
    if "nc" not in _COMPILED:
        _COMPILED["nc"] = _build_program()
    return _COMPILED["nc"]


def _build_in_maps(A, B):
    import ml_dtypes

    A = np.asarray(A, dtype=np.float32)
    B = np.asarray(B, dtype=np.float32)
    assert A.shape == (M_FULL, N), A.shape
    assert B.shape == (N, N), B.shape

    Qs, etr, ebl = _build_spectral()
    bf = ml_dtypes.bfloat16

    a_t = np.ascontiguousarray(A.T.astype(bf))              # [4096, 8192]
    b_t = np.ascontiguousarray(B.T.astype(bf))              # [4096, 4096]
    qs = np.ascontiguousarray(Qs.astype(bf))                # [4096, 768]
    ec = np.ascontiguousarray(
        np.concatenate([-etr, -ebl], axis=1).astype(bf))    # [128, 256]
    # packed corner B.T rows: j<64 -> B.T[j] (for Ebl, support j<=53);
    # j>=64 -> B.T[3968+j] (for Etr, support j>=74)
    btc = np.concatenate([b_t[0:64, :], b_t[N - 64:N, :]], axis=0).astype(bf)

    # XOR-slot exchange: core r's hg slot k holds the H slice of core
    # r ^ F[k] (F = ucode lane map), so chunk k of its output covers
    # global columns (r ^ F[k]); permute btc to match (out un-permuted
    # on the host in kernel()).
    def perm_cols(mat, r):
        return np.concatenate(
            [mat[:, (r ^ F_LANE[k]) * CW:((r ^ F_LANE[k]) + 1) * CW]
             for k in range(NCORES)], axis=1)

    return [
        {
            "at": np.ascontiguousarray(a_t[:, c * MS:(c + 1) * MS]),
            "btsl": np.ascontiguousarray(b_t[:, c * CW:(c + 1) * CW]),
            "qs": qs,
            "ec": ec,
            "btc": np.ascontiguousarray(perm_cols(btc, c)),
        }
        for c in range(NCORES)
    ]


def kernel(A, B):
    in_maps = _build_in_maps(A, B)
    nc = _get_program()
    res = run_bass_kernel_spmd(nc, in_maps, core_ids=list(range(NCORES)))
    out = np.empty((M_FULL, N), dtype=np.float32)
    for c in range(NCORES):
        oc = np.asarray(res.results[c]["out"]).astype(np.float32)
        for k in range(NCORES):
            g = c ^ F_LANE[k]
            out[c * MS:(c + 1) * MS, g * CW:(g + 1) * CW] = \
                oc[:, k * CW:(k + 1) * CW]
    return out


# revision 20
# speedup vs baseline: 9.2902x; 9.2902x over previous
"""Trainium2 kernel for out = A @ W2 @ B.T with banded Gaussian W2.

Math: W2 = W1*W1, W1[i,j] = exp(-(i-j)^2/(2*8^2)) truncated below 1e-10, so
W2 = exp(-(i-j)^2/64) on a |i-j| <= 54 band. W2 is symmetric banded Toeplitz.

Embed W2 in the 4096-circulant C with the same band symbol:
    C = W2 + E,  E = two 54x54 corner triangles (the wrap-around band).
C diagonalizes in the real DFT basis: C = Q diag(lam) Q.T with
lam(f) ~ sqrt(64*pi) * exp(-16 * (2*pi*f/N)^2). All kept lam > 0, so with
Qs = Q * sqrt(lam) (columns ordered by frequency: dc, c1, s1, c2, s2, ...)
and rank R = 768 (trunc err ~4e-3; the seed-0 inputs are spectrally
tilted, so R=640's cutoff at f=320 costs 1.9e-2 -- too close to the gate):

    out = (A @ Qs) @ (B @ Qs).T - A @ E @ B.T

One basis matrix serves both projections (half the Q traffic of a
Q / Q*lam split).

Distribution (8 cores, XOR-slot remote_dma exchange, no AllGather):
  - core c holds: A.T slab [4096, 1024] (its 1024 A-rows), B.T column slice
    [4096, 512], Qs [4096, 768], corner packs (all bf16).
  - stage H (sharded): HT_loc = Qs.T @ B.T[:, c-slice] [768, 512] written to
    SBUF slot 0 of hg, then remote_dma_broadcast SBUF->SBUF to peer
    (c ^ F_LANE[k])'s slot k -- no HBM bounce, no data collective. A 32-byte
    AllGather after trigger_dma is the rendezvous barrier; its latency plus
    stage G (~70us of PE work) covers the transfer drain. Final-stage chunk
    k = global column chunk c ^ F_LANE[k]; the host permutes btc in and
    un-permutes out. Cross-exec safety: executions serialize per core, and
    a peer's next-exec sends happen only after its rendezvous, which needs
    every core to have entered the next exec.
  - stage G (data-parallel): GT = Qs.T @ A.T_slab [768, 1024] in two m-half
    passes (PSUM: 6 f-banks + 1 corner bank per pass), A.T streamed.
  - corner factor: GTc = (-Etr).T @ A.T[x=0] + (-Ebl).T @ A.T[x=31],
    accumulated inside the stage-G passes.
  - final: out_slab = GT.T @ HT + GTc.T @ B.T_corner, streamed per 512-col
    chunk with all 8 PSUM banks accumulating the 8 m-tiles.
  - matmuls bf16 (1 cyc/row, FP32 PSUM accumulate), except the final
    GEMM's two highest f-tiles (f >= 256, negligible lam^2 weight) which
    run as one fp8e4 DoubleRow matmul (2 k-rows/cycle); their quantize
    copies are gated one chunk behind the PE-paced final stage so remote
    slots are never read before the SDMA data lands. out stored bf16,
    upcast on host. Measured rel err 6.7e-3 < 2e-2.
"""

import numpy as np

import concourse.bass as bass
import concourse.mybir as mybir
from concourse import bacc
from concourse.bass_utils import run_bass_kernel_spmd
from concourse.tile import TileContext
from concourse.tile_rust import add_dep_helper

P = 128          # partition / block size
N = 4096         # inner dims (A cols, B rows/cols)
M_FULL = 8192    # A rows
NCORES = 8
MS = M_FULL // NCORES   # 1024 rows of A per core
NK = N // P      # 32 contraction x-tiles
NM = MS // P     # 8 m-tiles per core
CW = 512         # output column chunk width (= 1 PSUM bank of fp32)
NCH = N // CW    # 8 chunks
R = 768          # truncated spectral rank (6 f-tiles)
NT = R // P      # 6 f-tiles

SIGMA = 8.0
TRUNC_EPS = 1e-10
# remote_dma_broadcast slot k applies delta-tpb F_LANE[k] (ucode lane map,
# measured on hardware): slots 4-7 swap pairs.
F_LANE = [0, 1, 2, 3, 6, 7, 4, 5]

_COMPILED = {}


def _band_profile():
    """g[d] = W2 band value at distance d (same fp32 path as the reference)."""
    d = np.arange(N).astype(np.float32)
    w1 = np.exp(-(d * d) / np.float32(2.0 * SIGMA * SIGMA)).astype(np.float32)
    w1 = np.where(w1 > np.float32(TRUNC_EPS), w1, np.float32(0.0))
    return (w1 * w1).astype(np.float64)


def _build_spectral():
    """Qs [N, R] freq-interleaved real-DFT basis * sqrt(lam), corner packs."""
    g = _band_profile()
    # circulant symbol c[k] = g(k) + g(N-k)
    c = g.copy()
    c[1:] += g[1:][::-1]
    lam = np.fft.rfft(c).real  # f = 0 .. N/2

    x = np.arange(N).astype(np.float64)
    cols = [np.full(N, 1.0 / np.sqrt(N))]
    lams = [lam[0]]
    f = 1
    while len(cols) < R:
        cols.append(np.sqrt(2.0 / N) * np.cos(2 * np.pi * f * x / N))
        lams.append(lam[f])
        if len(cols) < R:
            cols.append(np.sqrt(2.0 / N) * np.sin(2 * np.pi * f * x / N))
            lams.append(lam[f])
        f += 1
    Q = np.stack(cols, axis=1)                      # [N, R]
    lams = np.array(lams)
    assert (lams > 0).all(), lams.min()
    Qs = Q * np.sqrt(lams)[None, :]

    # corner blocks of E = C - W2 (W2 is zero there):
    # Etr[i, j] = c[(i - j - (N - P)) mod N] = g(i + P - j) for i+P-j <= 54
    ii = np.arange(P)[:, None]
    jj = np.arange(P)[None, :]
    dtr = ii + P - jj
    etr = np.where((dtr >= 0) & (dtr <= 54), g[np.clip(dtr, 0, 54)], 0.0)
    ebl = etr.T
    return Qs.astype(np.float32), etr.astype(np.float32), ebl.astype(np.float32)


def _build_program(reps=1):
    """Build + compile the Bass program (one NEFF, run SPMD on 8 cores)."""
    nc = bacc.Bacc("TRN2", target_bir_lowering=False, debug=False,
                   num_devices=NCORES)
    f32 = mybir.dt.float32
    bf16 = mybir.dt.bfloat16
    f8 = mybir.dt.float8e4

    at_dram = nc.dram_tensor("at", [N, MS], bf16, kind="ExternalInput").ap()
    btsl_dram = nc.dram_tensor("btsl", [N, CW], bf16,
                               kind="ExternalInput").ap()
    qs_dram = nc.dram_tensor("qs", [N, R], bf16, kind="ExternalInput").ap()
    ec_dram = nc.dram_tensor("ec", [P, 2 * P], bf16, kind="ExternalInput").ap()
    btc_dram = nc.dram_tensor("btc", [P, N], bf16,
                              kind="ExternalInput").ap()
    out_dram = nc.dram_tensor("out", [MS, N], bf16, kind="ExternalOutput").ap()

    at3 = at_dram.rearrange("(x p) m -> p x m", p=P)      # [128, 32, 1024]
    btsl3 = btsl_dram.rearrange("(x p) c -> p x c", p=P)  # [128, 32, 512]
    qs3 = qs_dram.rearrange("(x p) f -> p x f", p=P)      # [128, 32, 768]
    out3 = out_dram.rearrange("(m p) c -> p m c", p=P)    # [128, 8, 4096]

    rsem = nc.alloc_semaphore("rdma_data")
    lsem = nc.alloc_semaphore("rdma_local")

    with TileContext(nc) as tc:
        with (
            tc.tile_pool(name="const", bufs=1) as const_pool,
            tc.tile_pool(name="resp", bufs=1) as res_pool,
            tc.tile_pool(name="astr", bufs=3) as a_pool,
            tc.tile_pool(name="obw", bufs=2) as obw_pool,
            tc.tile_pool(name="psp", bufs=8, space="PSUM") as ps_pool,
        ):
            for rep in range(reps):
                sfx = f"_{rep}"
                # tiny DRAM flag tensors for the rendezvous collectives
                fl_in = [nc.dram_tensor(f"fl{i}{sfx}", [1, 16], bf16,
                                        kind="Internal").ap()
                         for i in range(1)]
                fl_out = [nc.dram_tensor(f"flg{i}{sfx}", [NCORES, 16], bf16,
                                         kind="Internal",
                                         addr_space="Shared").ap()
                          for i in range(1)]

                # ---- resident loads for stage H, interleaved per 4-x group
                # so x-tile k of both qs and btsl arrives in order.
                qs_sb = res_pool.tile([P, NK * R], bf16, tag="qs",
                                      name=f"qs_sb{sfx}")
                qs_v = qs_sb.rearrange("p (x f) -> p x f", f=R)
                btsl_sb = res_pool.tile([P, NK * CW], bf16, tag="btsl",
                                        name=f"btsl_sb{sfx}")
                btsl_v = btsl_sb.rearrange("p (x c) -> p x c", c=CW)
                QG = 4  # x-tiles per group DMA (steady state)
                # leading groups split small so the first H matmul starts
                # ~1.5us in instead of waiting behind a 1.2MB group
                edges = [0, 1, 2, 4, 8] + list(range(12, NK + 1, QG))
                for lo, hi in zip(edges, edges[1:]):
                    sl = slice(lo, hi)
                    nc.sync.dma_start(qs_v[:, sl, :], qs3[:, sl, :])
                    nc.sync.dma_start(btsl_v[:, sl, :], btsl3[:, sl, :])

                # ---- stage H: HT_loc = qs.T @ btsl -> hg slot 0
                ps_h = [
                    ps_pool.tile([P, CW], f32, tag="ps", name=f"ps_h{sfx}_{t}")
                    for t in range(NT)
                ]
                for x in range(NK):
                    for t in range(NT):
                        nc.tensor.matmul(
                            ps_h[t],
                            lhsT=qs_sb[:, x * R + t * P:x * R + (t + 1) * P],
                            rhs=btsl_sb[:, x * CW:(x + 1) * CW],
                            start=(x == 0),
                            stop=(x == NK - 1),
                        )
                # hg holds all 8 cores' H slices in SBUF, slot-ordered:
                # slot k of core r = H columns of core r ^ F[k] (host
                # un-permutes). Slot 0 is the local slice, written directly.
                hg_sb = res_pool.tile([P, NCORES * NT * CW], bf16, tag="hg",
                                      name=f"hg_sb{sfx}")
                for t in range(NT):
                    nc.vector.tensor_copy(
                        hg_sb[:, t * CW:(t + 1) * CW], ps_h[t])
                # broadcast slot 0 to peer r^F[k]'s slot k, SBUF -> SBUF
                sw = NT * CW
                for k in range(1, NCORES):
                    rdests = [(0, k) if j == k else None
                              for j in range(NCORES)]
                    nc.gpsimd.remote_dma_broadcast(
                        hg_sb[:, k * sw:(k + 1) * sw],
                        hg_sb[:, 0:sw],
                        remote_sem=rsem, local_sem=lsem, rdests=rdests,
                    )
                trig = nc.gpsimd.trigger_dma(count=None)
                # rendezvous AFTER the trigger (explicit dep: the scheduler
                # would otherwise hoist the dep-free collective to t=0):
                # barrier completion => every core rang its SWDGE doorbell,
                # and the barrier latency plus stage G (~70us) covers the
                # transfer drain before the first remote-slot read.
                cc = nc.gpsimd.collective_compute(
                    "AllGather",
                    mybir.AluOpType.bypass,
                    replica_groups=[list(range(NCORES))],
                    ins=[fl_in[0]],
                    outs=[fl_out[0]],
                )
                add_dep_helper(cc.ins, trig.ins,
                               reason="rendezvous after doorbell")

                # constants for stage G corners + final
                ec_sb = const_pool.tile([P, 2 * P], bf16, tag="ec",
                                        name=f"ec_sb{sfx}")
                nc.sync.dma_start(ec_sb, ec_dram)
                btc_sb = const_pool.tile([P, N], bf16, tag="btc",
                                         name=f"btc_sb{sfx}")
                nc.sync.dma_start(btc_sb, btc_dram)

                # ---- stage G: GT = qs.T @ at, two m-half passes + corner
                gt_sb = [
                    res_pool.tile([P, MS], bf16, tag=f"gt{t}",
                                  name=f"gt_sb{sfx}_{t}")
                    for t in range(NT)
                ]
                gtc_sb = res_pool.tile([P, MS], bf16, tag="gtc",
                                       name=f"gtc_sb{sfx}")
                # f-tiles 4,5 (f >= 256, tiny lam^2 weight) run the final
                # GEMM in fp8e4 DoubleRow (2 k-rows/cycle): G quantized
                # straight from PSUM, H re-quantized from the bf16 arrivals.
                # layout [128, (m, kt, P)] so lhsT slices are [128, 2, 128]
                gt45_f8 = res_pool.tile([P, NM * 2 * P], f8,
                                        tag="gt45", name=f"gt45{sfx}")
                for mh in range(2):
                    msl = slice(mh * CW, (mh + 1) * CW)
                    ps_g = [
                        ps_pool.tile([P, CW], f32, tag="ps",
                                     name=f"ps_g{sfx}_{mh}_{t}")
                        for t in range(NT)
                    ]
                    ps_c = ps_pool.tile([P, CW], f32, tag="ps",
                                        name=f"ps_c{sfx}_{mh}")
                    for xg in range(NK // QG):
                        at_t = a_pool.tile([P, QG * CW], bf16, tag="at",
                                           name=f"at_sb{sfx}_{mh}_{xg}")
                        nc.sync.dma_start(
                            at_t.rearrange("p (x m) -> p x m", m=CW),
                            at3[:, xg * QG:(xg + 1) * QG, msl])
                        for xi in range(QG):
                            x = xg * QG + xi
                            rhs = at_t[:, xi * CW:(xi + 1) * CW]
                            for t in range(NT):
                                nc.tensor.matmul(
                                    ps_g[t],
                                    lhsT=qs_sb[:, x * R + t * P:
                                               x * R + (t + 1) * P],
                                    rhs=rhs,
                                    start=(x == 0),
                                    stop=(x == NK - 1),
                                )
                            if x == 0:
                                nc.tensor.matmul(
                                    ps_c, lhsT=ec_sb[:, 0:P], rhs=rhs,
                                    start=True, stop=False)
                            elif x == NK - 1:
                                nc.tensor.matmul(
                                    ps_c, lhsT=ec_sb[:, P:2 * P], rhs=rhs,
                                    start=False, stop=True)
                    for t in range(NT - 2):
                        nc.vector.tensor_copy(gt_sb[t][:, msl], ps_g[t])
                    for t in (NT - 2, NT - 1):
                        for j in range(NM // 2):
                            m = mh * (NM // 2) + j
                            dst = (m * 2 + (t - (NT - 2))) * P
                            nc.scalar.activation(
                                gt45_f8[:, dst:dst + P],
                                ps_g[t][:, j * P:(j + 1) * P],
                                mybir.ActivationFunctionType.Copy)
                    nc.vector.tensor_copy(gtc_sb[:, msl], ps_c)

                # ---- final: out = GT.T @ HT + GTc.T @ btc, per 512-col chunk
                hn45_f8 = res_pool.tile([P, NCH * 2 * CW], f8,
                                        tag="hn45", name=f"hn45{sfx}")
                prev_first = None
                for nu in range(NCH):
                    cs = bass.ts(nu, CW)
                    hn_t = hg_sb[:, nu * NT * CW:(nu + 1) * NT * CW]
                    # quantize this chunk's H tiles 4,5 to fp8. Remote slots
                    # (nu>0) must not be read before the SDMA data lands:
                    # gate on the PREVIOUS chunk's first matmul, which runs
                    # ~12us/chunk behind the PE-paced final stage, far after
                    # the ~85us arrival time (the bf16 matmuls get the same
                    # safety implicitly from PE program order).
                    for t in (NT - 2, NT - 1):
                        cp = nc.scalar.activation(
                            hn45_f8[:, (nu * 2 + t - (NT - 2)) * CW:
                                    (nu * 2 + t - (NT - 2) + 1) * CW],
                            hn_t[:, t * CW:(t + 1) * CW],
                            mybir.ActivationFunctionType.Copy)
                        if nu > 0:
                            add_dep_helper(cp.ins, cc.ins,
                                           reason="quantize after barrier")
                            if prev_first is not None:
                                add_dep_helper(cp.ins, prev_first.ins,
                                               reason="quantize after arrival")
                    ps_o = [
                        ps_pool.tile([P, CW], f32, tag="ps",
                                     name=f"ps_o{sfx}_{nu}_{m}")
                        for m in range(NM)
                    ]
                    obw_t = obw_pool.tile([P, NM * CW], bf16, tag="obw",
                                          name=f"obw_sb{sfx}_{nu}")
                    # m-outer with per-m PSUM copy: each m's copy overlaps
                    # the next m's matmuls, shrinking the chunk drain tail
                    for m in range(NM):
                        nc.tensor.matmul(
                            ps_o[m],
                            lhsT=gtc_sb[:, m * P:(m + 1) * P],
                            rhs=btc_sb[:, cs],
                            start=True,
                            stop=False,
                        )
                        for t in range(NT - 2):
                            mm = nc.tensor.matmul(
                                ps_o[m],
                                lhsT=gt_sb[t][:, m * P:(m + 1) * P],
                                rhs=hn_t[:, t * CW:(t + 1) * CW],
                                start=False,
                                stop=False,
                            )
                            if m == 0 and t == 0:
                                if nu > 0:
                                    add_dep_helper(
                                        mm.ins, cc.ins,
                                        reason="remote H after barrier")
                                prev_first = mm
                        nc.tensor.matmul(
                            ps_o[m],
                            lhsT=gt45_f8.rearrange(
                                "p (m kt q) -> p m kt q", kt=2, q=P)[:, m],
                            rhs=hn45_f8.rearrange(
                                "p (u kt c) -> p u kt c", kt=2, c=CW)[:, nu],
                            start=False,
                            stop=True,
                            perf_mode=mybir.MatmulPerfMode.DoubleRow,
                        )
                        nc.vector.tensor_copy(
                            obw_t[:, m * CW:(m + 1) * CW], ps_o[m])
                    nc.sync.dma_start(
                        out3[:, :, cs],
                        obw_t.rearrange("p (m c) -> p m c", c=CW),
                    )

    nc.compile()
    return nc


def _get_program():
    if "nc" not in _COMPILED:
        _COMPILED["nc"] = _build_program()
    return _COMPILED["nc"]


def _build_in_maps(A, B):
    import ml_dtypes

    A = np.asarray(A, dtype=np.float32)
    B = np.asarray(B, dtype=np.float32)
    assert A.shape == (M_FULL, N), A.shape
    assert B.shape == (N, N), B.shape

    Qs, etr, ebl = _build_spectral()
    bf = ml_dtypes.bfloat16

    a_t = np.ascontiguousarray(A.T.astype(bf))              # [4096, 8192]
    b_t = np.ascontiguousarray(B.T.astype(bf))              # [4096, 4096]
    qs = np.ascontiguousarray(Qs.astype(bf))                # [4096, 768]
    ec = np.ascontiguousarray(
        np.concatenate([-etr, -ebl], axis=1).astype(bf))    # [128, 256]
    # packed corner B.T rows: j<64 -> B.T[j] (for Ebl, support j<=53);
    # j>=64 -> B.T[3968+j] (for Etr, support j>=74)
    btc = np.concatenate([b_t[0:64, :], b_t[N - 64:N, :]], axis=0).astype(bf)

    # XOR-slot exchange: core r's hg slot k holds the H slice of core
    # r ^ F[k] (F = ucode lane map), so chunk k of its output covers
    # global columns (r ^ F[k]); permute btc to match (out un-permuted
    # on the host in kernel()).
    def perm_cols(mat, r):
        return np.concatenate(
            [mat[:, (r ^ F_LANE[k]) * CW:((r ^ F_LANE[k]) + 1) * CW]
             for k in range(NCORES)], axis=1)

    return [
        {
            "at": np.ascontiguousarray(a_t[:, c * MS:(c + 1) * MS]),
            "btsl": np.ascontiguousarray(b_t[:, c * CW:(c + 1) * CW]),
            "qs": qs,
            "ec": ec,
            "btc": np.ascontiguousarray(perm_cols(btc, c)),
        }
        for c in range(NCORES)
    ]


def kernel(A, B):
    in_maps = _build_in_maps(A, B)
    nc = _get_program()
    res = run_bass_kernel_spmd(nc, in_maps, core_ids=list(range(NCORES)))
    out = np.empty((M_FULL, N), dtype=np.float32)
    for c in range(NCORES):
        oc = np.asarray(res.results[c]["out"]).astype(np.float32)
        for k in range(NCORES):
            g = c ^ F_LANE[k]
            out[c * MS:(c + 1) * MS, g * CW:(g + 1) * CW] = \
                oc[:, k * CW:(k + 1) * CW]
    return out
